# revision 6
# baseline (speedup 1.0000x reference)
"""Trainium2 Bass kernel for the BoundaryPredictor module.

Contract: kernel(**inputs) takes the FULL unsharded inputs (numpy arrays,
keys as in setup_inputs) and returns the full output tuple
(pooled[B,L,D], loss, hb_sum, n_sum, short_mask[B,L]).

Sharding: data-parallel over batch B across the 8 NeuronCores (one batch
row per core). Everything per-row runs on device; only the final scalar
loss / sums are assembled on host from tiny per-row device outputs.
"""

import os
import sys

import numpy as np

for _p in ("/opt/trn_rl_repo",):
    if os.path.isdir(_p) and _p not in sys.path:
        sys.path.insert(0, _p)

B, L, D, H = 8, 2048, 512, 2048
P = 128
NI = L // P      # 16 token tiles (layout A: l = i*128 + p)
KD = D // P      # 4 contraction tiles over D
NH = H // P      # 16 tiles over H
CH = 512         # token chunk for the MLP moving dim
NCH = L // CH    # 4 chunks
NS = NI          # 16 segment tiles
TEMP, THRESHOLD, PRIOR = 1.0, 0.5, 0.2
EPS = 1e-6

_NC_CACHE = {}


def _build_nc(pool_dt_name="float32"):
    import concourse.bacc as bacc
    import concourse.tile as tile
    from concourse import mybir
    from concourse.masks import make_identity, make_upper_triangular

    dt = mybir.dt
    Alu = mybir.AluOpType
    Act = mybir.ActivationFunctionType
    f32 = dt.float32
    pool_dt = getattr(dt, pool_dt_name)

    nc = bacc.Bacc()

    hid_d = nc.dram_tensor("hidden", [L, D], f32, kind="ExternalInput")
    mask_d = nc.dram_tensor("mask", [L], f32, kind="ExternalInput")
    u_d = nc.dram_tensor("u", [L], f32, kind="ExternalInput")
    W1_d = nc.dram_tensor("W1", [D, H], f32, kind="ExternalInput")
    b1_d = nc.dram_tensor("b1", [H], f32, kind="ExternalInput")
    W2_d = nc.dram_tensor("W2", [H, 1], f32, kind="ExternalInput")
    b2_d = nc.dram_tensor("b2", [1], f32, kind="ExternalInput")

    pooled_d = nc.dram_tensor("pooled", [L, D], f32, kind="ExternalOutput")
    short_d = nc.dram_tensor("short_mask", [L], f32, kind="ExternalOutput")
    counts_d = nc.dram_tensor("counts", [1], f32, kind="ExternalOutput")

    with tile.TileContext(nc) as tc:
        with (
            tc.tile_pool(name="const", bufs=1) as cpool,
            tc.tile_pool(name="big", bufs=1) as bpool,
            tc.tile_pool(name="small", bufs=1) as spool,
        ):
            # ---- constants ----
            ident = cpool.tile([P, P], f32)
            make_identity(nc, ident[:])
            # ltri[k, m] = 1 iff k < m  -> matmul(ltri, x) = exclusive prefix
            # over the partition dim
            ltri = cpool.tile([P, P], f32)
            make_upper_triangular(nc, ltri[:], val=1.0, diag=False)
            ones_col = cpool.tile([P, 1], f32)
            nc.vector.memset(ones_col[:], 1.0)
            ones_row = cpool.tile([1, P], f32)
            nc.vector.memset(ones_row[:], 1.0)
            iotaB_i = cpool.tile([P, P], dt.int32)
            nc.gpsimd.iota(iotaB_i[:], pattern=[[1, P]], base=0, channel_multiplier=0)
            iotaB = cpool.tile([P, P], f32)
            nc.vector.tensor_copy(iotaB[:], iotaB_i[:])
            iotaP_i = cpool.tile([P, NI], dt.int32)
            nc.gpsimd.iota(iotaP_i[:], pattern=[[P, NI]], base=0, channel_multiplier=1)
            iotaP = cpool.tile([P, NI], f32)
            nc.vector.tensor_copy(iotaP[:], iotaP_i[:])

            # ---- big persistent inputs ----
            W1_sb = bpool.tile([P, KD, H], f32)
            W1_r = W1_d.rearrange("(k p) h -> p k h", p=P)
            for k in range(KD):
                nc.sync.dma_start(W1_sb[:, k, :], W1_r[:, k, :])
            # tokens in layout A (l = i*128 + p), plus a ones column at D for
            # the per-segment count histogram
            h_sb = bpool.tile([P, NI, D + 1], pool_dt)
            hid_r = hid_d.rearrange("(i p) d -> p i d", p=P)
            for i in range(NI):
                nc.sync.dma_start(h_sb[:, i, 0:D], hid_r[:, i, :])
            nc.vector.memset(h_sb[:, :, D : D + 1], 1.0)
            hT = bpool.tile([P, KD, L], f32)  # hT[pd, k, t] = hidden[t, k*128+pd]

            b1_sb = spool.tile([P, NH], f32)
            nc.sync.dma_start(b1_sb[:], b1_d.rearrange("(i p) -> p i", p=P))
            W2_sb = spool.tile([P, NH], f32)
            nc.sync.dma_start(W2_sb[:], W2_d.rearrange("(i p) one -> p (i one)", p=P))
            b2_sb = spool.tile([1, 1], f32)
            nc.sync.dma_start(b2_sb[:], b2_d.rearrange("(a b) -> a b", a=1))
            uP = spool.tile([P, NI], f32)
            nc.sync.dma_start(uP[:], u_d.rearrange("(i p) -> p i", p=P))
            maskP = spool.tile([P, NI], f32)
            nc.sync.dma_start(maskP[:], mask_d.rearrange("(i p) -> p i", p=P))
            logitsP = spool.tile([P, NI], f32)
            b2b = spool.tile([P, 1], f32)

            # ================= phase 1+2: transpose + MLP =================
            with (
                tc.tile_pool(name="ps_tr", bufs=2, space="PSUM") as ps_tr,
                tc.tile_pool(name="ps_mlp", bufs=2, space="PSUM") as ps_mlp,
                tc.tile_pool(name="ps_sm", bufs=2, space="PSUM") as ps_sm,
                tc.tile_pool(name="actp", bufs=3) as actp,
                tc.tile_pool(name="laccp", bufs=2) as laccp,
            ):
                b2ps = ps_sm.tile([P, 1], f32)
                nc.tensor.matmul(b2ps[:], ones_row[:], b2_sb[:], start=True, stop=True)
                nc.vector.tensor_copy(b2b[:], b2ps[:])

                for i in range(NI):
                    for k in range(KD):
                        pst = ps_tr.tile([P, P], f32)
                        nc.tensor.transpose(
                            pst[:], h_sb[:, i, k * P : (k + 1) * P], ident[:]
                        )
                        nc.scalar.copy(out=hT[:, k, i * P : (i + 1) * P], in_=pst[:])

                for c in range(NCH):
                    lacc = laccp.tile([P, CH], f32)
                    for ht in range(NH):
                        pm = ps_mlp.tile([P, CH], f32)
                        for k in range(KD):
                            nc.tensor.matmul(
                                pm[:],
                                W1_sb[:, k, ht * P : (ht + 1) * P],
                                hT[:, k, c * CH : (c + 1) * CH],
                                start=(k == 0),
                                stop=(k == KD - 1),
                            )
                        at = actp.tile([P, CH], f32)
                        nc.scalar.activation(
                            at[:], pm[:], Act.Relu, bias=b1_sb[:, ht : ht + 1], scale=1.0
                        )
                        if ht == 0:
                            nc.vector.tensor_scalar(
                                lacc[:], at[:], W2_sb[:, 0:1], None, op0=Alu.mult
                            )
                        else:
                            nc.vector.scalar_tensor_tensor(
                                out=lacc[:],
                                in0=at[:],
                                scalar=W2_sb[:, ht : ht + 1],
                                in1=lacc[:],
                                op0=Alu.mult,
                                op1=Alu.add,
                            )
                    for j in range(CH // P):
                        pl = ps_sm.tile([P, 1], f32)
                        nc.tensor.matmul(
                            pl[:], lacc[:, j * P : (j + 1) * P], ones_col[:],
                            start=True, stop=True,
                        )
                        col = c * (CH // P) + j
                        nc.vector.tensor_scalar(
                            logitsP[:, col : col + 1], pl[:], b2b[:], None,
                            op0=Alu.add,
                        )

            # ================= phase 3: sampling + segment ids ============
            segP = spool.tile([P, NI], f32)
            counts_sb = spool.tile([1, 1], f32)
            with (
                tc.tile_pool(name="ps_row", bufs=1, space="PSUM") as ps_row,
                tc.tile_pool(name="ps_seg", bufs=1, space="PSUM") as ps_seg,
                tc.tile_pool(name="ph3", bufs=1) as ph3,
            ):
                ln_u = ph3.tile([P, NI], f32)
                nc.scalar.activation(ln_u[:], uP[:], Act.Ln)
                omu = ph3.tile([P, NI], f32)
                nc.vector.tensor_scalar(
                    omu[:], uP[:], -1.0, 1.0, op0=Alu.mult, op1=Alu.add
                )
                ln_omu = ph3.tile([P, NI], f32)
                nc.scalar.activation(ln_omu[:], omu[:], Act.Ln)
                noiseP = ph3.tile([P, NI], f32)
                nc.vector.tensor_sub(noiseP[:], ln_u[:], ln_omu[:])
                xP = ph3.tile([P, NI], f32)
                nc.vector.tensor_add(xP[:], logitsP[:], noiseP[:])
                hard = ph3.tile([P, NI], f32)
                nc.vector.tensor_scalar(hard[:], xP[:], 0.0, None, op0=Alu.is_gt)
                hb0 = ph3.tile([P, NI], f32)
                nc.vector.tensor_mul(hb0[:], hard[:], maskP[:])
                padP = ph3.tile([P, NI], f32)
                nc.vector.tensor_scalar(
                    padP[:], maskP[:], -1.0, 1.0, op0=Alu.mult, op1=Alu.add
                )

                def excl_prefix(src, tag):
                    cs_ps = ps_row.tile([1, NI], f32, tag=f"cs_{tag}")
                    nc.tensor.matmul(cs_ps[:], ones_col[:], src[:], start=True, stop=True)
                    cs = ph3.tile([1, NI], f32, tag=f"cssb_{tag}")
                    nc.vector.tensor_copy(cs[:], cs_ps[:])
                    inc = ph3.tile([1, NI], f32, tag=f"inc_{tag}")
                    nc.vector.tensor_tensor_scan(
                        inc[:], cs[:], cs[:], 0.0, op0=Alu.add, op1=Alu.bypass
                    )
                    exc = ph3.tile([1, NI], f32, tag=f"exc_{tag}")
                    nc.vector.tensor_sub(exc[:], inc[:], cs[:])
                    pfx = ps_seg.tile([P, NI], f32, tag=f"pfx_{tag}")
                    nc.tensor.matmul(pfx[:], ltri[:], src[:], start=True, stop=False)
                    nc.tensor.matmul(pfx[:], ones_row[:], exc[:], start=False, stop=True)
                    return pfx, inc

                pfx_pad, _ = excl_prefix(padP, "pad")
                inclp = ph3.tile([P, NI], f32)
                nc.vector.tensor_add(inclp[:], pfx_pad[:], padP[:])
                fp_t = ph3.tile([P, NI], f32)
                nc.vector.scalar_tensor_tensor(
                    out=fp_t[:], in0=inclp[:], scalar=ones_col[:], in1=padP[:],
                    op0=Alu.is_equal, op1=Alu.mult,
                )
                lr = ph3.tile([P, NI], f32)
                nc.vector.memset(lr[:], 0.0)
                nc.sync.dma_start(lr[0 : P - 1, :], fp_t[1:P, :])
                nc.sync.dma_start(lr[P - 1 : P, 0 : NI - 1], fp_t[0:1, 1:NI])
                hbP = ph3.tile([P, NI], f32)
                nc.vector.tensor_max(hbP[:], hb0[:], lr[:])

                pfx_hb, inc_hb = excl_prefix(hbP, "hb")
                nc.vector.tensor_copy(segP[:], pfx_hb[:])
                nc.vector.tensor_copy(counts_sb[:], inc_hb[0:1, NI - 1 : NI])
                nc.sync.dma_start(
                    counts_d.rearrange("(a b) -> a b", a=1), counts_sb[:]
                )
                cb_ps = ps_seg.tile([P, 1], f32)
                nc.tensor.matmul(
                    cb_ps[:], ones_row[:], counts_sb[:], start=True, stop=True
                )
                counts_b = ph3.tile([P, 1], f32)
                nc.vector.tensor_copy(counts_b[:], cb_ps[:])
                shortP = ph3.tile([P, NI], f32)
                nc.vector.tensor_scalar(
                    shortP[:], iotaP[:], counts_b[:], None, op0=Alu.is_lt
                )
                nc.sync.dma_start(
                    short_d.rearrange("(i p) -> p i", p=P), shortP[:]
                )

            # ================= phase 4: pooling ===========================
            with (
                tc.tile_pool(name="ps_pa", bufs=2, space="PSUM") as ps_pa,
                tc.tile_pool(name="ps_pb", bufs=2, space="PSUM") as ps_pb,
                tc.tile_pool(name="barp", bufs=4) as barp,
                tc.tile_pool(name="outp", bufs=3) as outp,
                tc.tile_pool(name="iop", bufs=3) as iop,
            ):
                for s in range(NS):
                    segS = iop.tile([P, NI], f32)
                    nc.vector.tensor_scalar(
                        segS[:], segP[:], float(s * P), None, op0=Alu.subtract
                    )
                    pa = ps_pa.tile([P, 256], f32)
                    pb = ps_pb.tile([P, 257], f32)
                    for i in range(NI):
                        bar = barp.tile([P, P], pool_dt)
                        nc.vector.tensor_scalar(
                            bar[:], iotaB[:], segS[:, i : i + 1], None,
                            op0=Alu.is_equal,
                        )
                        nc.tensor.matmul(
                            pa[:], bar[:], h_sb[:, i, 0:256],
                            start=(i == 0), stop=(i == NI - 1),
                        )
                        nc.tensor.matmul(
                            pb[:], bar[:], h_sb[:, i, 256 : D + 1],
                            start=(i == 0), stop=(i == NI - 1),
                        )
                    cnt_eps = iop.tile([P, 1], f32)
                    nc.vector.tensor_scalar(
                        cnt_eps[:], pb[:, 256:257], 1e-9, None, op0=Alu.add
                    )
                    invc = iop.tile([P, 1], f32)
                    nc.vector.reciprocal(invc[:], cnt_eps[:])
                    ot = outp.tile([P, D], f32)
                    nc.vector.tensor_scalar(
                        ot[:, 0:256], pa[:], invc[:], None, op0=Alu.mult
                    )
                    nc.vector.tensor_scalar(
                        ot[:, 256:512], pb[:, 0:256], invc[:], None, op0=Alu.mult
                    )
                    nc.sync.dma_start(pooled_d[s * P : (s + 1) * P, :], ot[:])

    nc.finalize()
    return nc


def get_nc(pool_dt_name="float32"):
    if pool_dt_name not in _NC_CACHE:
        _NC_CACHE[pool_dt_name] = _build_nc(pool_dt_name)
    return _NC_CACHE[pool_dt_name]


def _host_loss(counts, n, target_boundary_counts):
    from scipy.special import gammaln

    c = counts.astype(np.float64)
    n64 = n.astype(np.float64)
    p = np.clip(
        (target_boundary_counts.astype(np.float32) / n.astype(np.float32)).astype(
            np.float64
        ),
        EPS,
        1.0 - EPS,
    )
    logpmf = (
        gammaln(n64 + 1.0)
        - gammaln(c + 1.0)
        - gammaln(n64 - c + 1.0)
        + c * np.log(p)
        + (n64 - c) * np.log1p(-p)
    )
    return np.float32(np.mean(-logpmf))


def kernel(hidden, attention_mask, target_boundary_counts, W1, b1, W2, b2, u,
           _run=None, pool_dt_name="float32"):
    from concourse.bass_utils import run_bass_kernel_spmd

    hidden = np.ascontiguousarray(hidden, np.float32)
    attention_mask = np.ascontiguousarray(attention_mask, np.float32)
    u = np.ascontiguousarray(u, np.float32)
    W1 = np.ascontiguousarray(W1, np.float32)
    b1 = np.ascontiguousarray(b1, np.float32)
    W2 = np.ascontiguousarray(W2, np.float32)
    b2 = np.ascontiguousarray(b2, np.float32)

    nc = get_nc(pool_dt_name)
    in_maps = [
        dict(
            hidden=hidden[i], mask=attention_mask[i], u=u[i],
            W1=W1, b1=b1, W2=W2, b2=b2,
        )
        for i in range(B)
    ]
    if _run is None:
        results = run_bass_kernel_spmd(nc, in_maps, list(range(B))).results
    else:
        results = _run(nc, in_maps)

    pooled = np.stack([results[i]["pooled"] for i in range(B)])
    short_mask = np.stack([results[i]["short_mask"] for i in range(B)])
    counts = np.array([results[i]["counts"][0] for i in range(B)], np.float32)

    n = attention_mask.sum(axis=1, dtype=np.float32)
    loss = _host_loss(counts, n, target_boundary_counts)
    hb_sum = np.float32(counts.sum())
    n_sum = np.float32(n.sum())
    return pooled, loss, hb_sum, n_sum, short_mask


# revision 11
# speedup vs baseline: 1.4064x; 1.4064x over previous
"""Trainium2 Bass kernel for the BoundaryPredictor module.

Contract: kernel(**inputs) takes the FULL unsharded inputs (numpy arrays,
keys as in setup_inputs) and returns the full output tuple
(pooled[B,L,D], loss, hb_sum, n_sum, short_mask[B,L]).

Sharding: data-parallel over batch B across the 8 NeuronCores (one batch
row per core). Everything per-row runs on device; only the final scalar
loss / sums are assembled on host from tiny per-row device outputs.
"""

import os
import sys

import numpy as np

for _p in ("/opt/trn_rl_repo",):
    if os.path.isdir(_p) and _p not in sys.path:
        sys.path.insert(0, _p)

B, L, D, H = 8, 2048, 512, 2048
P = 128
NI = L // P      # 16 token tiles (layout A: l = i*128 + p)
KD = D // P      # 4 contraction tiles over D
NH = H // P      # 16 tiles over H
CH = 512         # token chunk for the MLP moving dim
NCH = L // CH    # 4 chunks
NS = NI          # 16 segment tiles
TEMP, THRESHOLD, PRIOR = 1.0, 0.5, 0.2
EPS = 1e-6

_NC_CACHE = {}


def _build_nc(pool_dt_name="float32"):
    import concourse.bacc as bacc
    import concourse.tile as tile
    from concourse import mybir
    from concourse.masks import make_identity, make_upper_triangular

    dt = mybir.dt
    Alu = mybir.AluOpType
    Act = mybir.ActivationFunctionType
    f32 = dt.float32
    pool_dt = getattr(dt, pool_dt_name)

    nc = bacc.Bacc()

    hid_d = nc.dram_tensor("hidden", [L, D], f32, kind="ExternalInput")
    mask_d = nc.dram_tensor("mask", [L], f32, kind="ExternalInput")
    u_d = nc.dram_tensor("u", [L], f32, kind="ExternalInput")
    W1_d = nc.dram_tensor("W1", [D, H], f32, kind="ExternalInput")
    b1_d = nc.dram_tensor("b1", [H], f32, kind="ExternalInput")
    W2_d = nc.dram_tensor("W2", [H, 1], f32, kind="ExternalInput")
    b2_d = nc.dram_tensor("b2", [1], f32, kind="ExternalInput")

    pooled_d = nc.dram_tensor("pooled", [L, D], f32, kind="ExternalOutput")
    short_d = nc.dram_tensor("short_mask", [L], f32, kind="ExternalOutput")
    counts_d = nc.dram_tensor("counts", [1], f32, kind="ExternalOutput")

    with tile.TileContext(nc) as tc:
        with (
            tc.tile_pool(name="const", bufs=1) as cpool,
            tc.tile_pool(name="big", bufs=1) as bpool,
            tc.tile_pool(name="small", bufs=1) as spool,
        ):
            # ---- constants ----
            ident = cpool.tile([P, P], f32)
            make_identity(nc, ident[:])
            # ltri[k, m] = 1 iff k < m  -> matmul(ltri, x) = exclusive prefix
            # over the partition dim
            ltri = cpool.tile([P, P], f32)
            make_upper_triangular(nc, ltri[:], val=1.0, diag=False)
            ones_col = cpool.tile([P, 1], f32)
            nc.vector.memset(ones_col[:], 1.0)
            ones_row = cpool.tile([1, P], f32)
            nc.vector.memset(ones_row[:], 1.0)
            iotaB_i = cpool.tile([P, P], dt.int32)
            nc.gpsimd.iota(iotaB_i[:], pattern=[[1, P]], base=0, channel_multiplier=0)
            iotaB = cpool.tile([P, P], f32)
            nc.vector.tensor_copy(iotaB[:], iotaB_i[:])
            iotaP_i = cpool.tile([P, NI], dt.int32)
            nc.gpsimd.iota(iotaP_i[:], pattern=[[P, NI]], base=0, channel_multiplier=1)
            iotaP = cpool.tile([P, NI], f32)
            nc.vector.tensor_copy(iotaP[:], iotaP_i[:])

            # ---- big persistent inputs ----
            W1_sb = bpool.tile([P, KD, H], f32)
            W1_r = W1_d.rearrange("(k p) h -> p k h", p=P)
            for k in range(KD):
                nc.sync.dma_start(W1_sb[:, k, :], W1_r[:, k, :])
            # tokens in layout A (l = i*128 + p), plus a ones column at D for
            # the per-segment count histogram
            h_sb = bpool.tile([P, NI, D + 2], f32)
            hid_r = hid_d.rearrange("(i p) d -> p i d", p=P)
            for i in range(NI):
                nc.sync.dma_start(h_sb[:, i, 0:D], hid_r[:, i, :])
            nc.vector.memset(h_sb[:, :, D : D + 2], 1.0)
            if pool_dt is f32:
                h_pool = h_sb
            else:
                # rounded copy for the reduced-precision pooling matmuls; the
                # f32 h_sb still feeds the precision-critical MLP transposes
                h_pool = bpool.tile([P, NI, D + 2], pool_dt)
                for i in range(NI):
                    nc.vector.tensor_copy(h_pool[:, i, :], h_sb[:, i, :])
            hT = bpool.tile([P, KD, L], f32)  # hT[pd, k, t] = hidden[t, k*128+pd]

            b1_sb = spool.tile([P, NH], f32)
            nc.sync.dma_start(b1_sb[:], b1_d.rearrange("(i p) -> p i", p=P))
            W2_sb = spool.tile([P, NH], f32)
            nc.sync.dma_start(W2_sb[:], W2_d.rearrange("(i p) one -> p (i one)", p=P))
            b2_sb = spool.tile([1, 1], f32)
            nc.sync.dma_start(b2_sb[:], b2_d.rearrange("(a b) -> a b", a=1))
            uP = spool.tile([P, NI], f32)
            nc.sync.dma_start(uP[:], u_d.rearrange("(i p) -> p i", p=P))
            maskP = spool.tile([P, NI], f32)
            nc.sync.dma_start(maskP[:], mask_d.rearrange("(i p) -> p i", p=P))
            logitsP = spool.tile([P, NI], f32)
            b2b = spool.tile([P, 1], f32)

            # ================= phase 1+2: transpose + MLP =================
            with (
                tc.tile_pool(name="ps_tr", bufs=2, space="PSUM") as ps_tr,
                tc.tile_pool(name="ps_mlp", bufs=2, space="PSUM") as ps_mlp,
                tc.tile_pool(name="ps_sm", bufs=2, space="PSUM") as ps_sm,
                tc.tile_pool(name="actp", bufs=3) as actp,
                tc.tile_pool(name="laccp", bufs=2) as laccp,
            ):
                b2ps = ps_sm.tile([P, 1], f32)
                nc.tensor.matmul(b2ps[:], ones_row[:], b2_sb[:], start=True, stop=True)
                nc.vector.tensor_copy(b2b[:], b2ps[:])

                for i in range(NI):
                    for k in range(KD):
                        pst = ps_tr.tile([P, P], f32)
                        nc.tensor.transpose(
                            pst[:], h_sb[:, i, k * P : (k + 1) * P], ident[:]
                        )
                        nc.scalar.copy(out=hT[:, k, i * P : (i + 1) * P], in_=pst[:])

                for c in range(NCH):
                    lacc = laccp.tile([P, CH], f32)
                    for ht in range(NH):
                        pm = ps_mlp.tile([P, CH], f32)
                        for k in range(KD):
                            nc.tensor.matmul(
                                pm[:],
                                W1_sb[:, k, ht * P : (ht + 1) * P],
                                hT[:, k, c * CH : (c + 1) * CH],
                                start=(k == 0),
                                stop=(k == KD - 1),
                            )
                        at = actp.tile([P, CH], f32)
                        nc.scalar.activation(
                            at[:], pm[:], Act.Relu, bias=b1_sb[:, ht : ht + 1], scale=1.0
                        )
                        if ht == 0:
                            nc.vector.tensor_scalar(
                                lacc[:], at[:], W2_sb[:, 0:1], None, op0=Alu.mult
                            )
                        else:
                            nc.vector.scalar_tensor_tensor(
                                out=lacc[:],
                                in0=at[:],
                                scalar=W2_sb[:, ht : ht + 1],
                                in1=lacc[:],
                                op0=Alu.mult,
                                op1=Alu.add,
                            )
                    for j in range(CH // P):
                        pl = ps_sm.tile([P, 1], f32)
                        nc.tensor.matmul(
                            pl[:], lacc[:, j * P : (j + 1) * P], ones_col[:],
                            start=True, stop=True,
                        )
                        col = c * (CH // P) + j
                        nc.vector.tensor_scalar(
                            logitsP[:, col : col + 1], pl[:], b2b[:], None,
                            op0=Alu.add,
                        )

            # ================= phase 3: sampling + segment ids ============
            segP = spool.tile([P, NI], f32)
            counts_sb = spool.tile([1, 1], f32)
            with (
                tc.tile_pool(name="ps_row", bufs=1, space="PSUM") as ps_row,
                tc.tile_pool(name="ps_seg", bufs=1, space="PSUM") as ps_seg,
                tc.tile_pool(name="ph3", bufs=1) as ph3,
            ):
                ln_u = ph3.tile([P, NI], f32)
                nc.scalar.activation(ln_u[:], uP[:], Act.Ln)
                omu = ph3.tile([P, NI], f32)
                nc.vector.tensor_scalar(
                    omu[:], uP[:], -1.0, 1.0, op0=Alu.mult, op1=Alu.add
                )
                ln_omu = ph3.tile([P, NI], f32)
                nc.scalar.activation(ln_omu[:], omu[:], Act.Ln)
                noiseP = ph3.tile([P, NI], f32)
                nc.vector.tensor_sub(noiseP[:], ln_u[:], ln_omu[:])
                xP = ph3.tile([P, NI], f32)
                nc.vector.tensor_add(xP[:], logitsP[:], noiseP[:])
                hard = ph3.tile([P, NI], f32)
                nc.vector.tensor_scalar(hard[:], xP[:], 0.0, None, op0=Alu.is_gt)
                hb0 = ph3.tile([P, NI], f32)
                nc.vector.tensor_mul(hb0[:], hard[:], maskP[:])
                padP = ph3.tile([P, NI], f32)
                nc.vector.tensor_scalar(
                    padP[:], maskP[:], -1.0, 1.0, op0=Alu.mult, op1=Alu.add
                )

                def excl_prefix(src, tag):
                    cs_ps = ps_row.tile([1, NI], f32, tag=f"cs_{tag}")
                    nc.tensor.matmul(cs_ps[:], ones_col[:], src[:], start=True, stop=True)
                    cs = ph3.tile([1, NI], f32, tag=f"cssb_{tag}")
                    nc.vector.tensor_copy(cs[:], cs_ps[:])
                    inc = ph3.tile([1, NI], f32, tag=f"inc_{tag}")
                    nc.vector.tensor_tensor_scan(
                        inc[:], cs[:], cs[:], 0.0, op0=Alu.add, op1=Alu.bypass
                    )
                    exc = ph3.tile([1, NI], f32, tag=f"exc_{tag}")
                    nc.vector.tensor_sub(exc[:], inc[:], cs[:])
                    pfx = ps_seg.tile([P, NI], f32, tag=f"pfx_{tag}")
                    nc.tensor.matmul(pfx[:], ltri[:], src[:], start=True, stop=False)
                    nc.tensor.matmul(pfx[:], ones_row[:], exc[:], start=False, stop=True)
                    return pfx, inc

                pfx_pad, _ = excl_prefix(padP, "pad")
                inclp = ph3.tile([P, NI], f32)
                nc.vector.tensor_add(inclp[:], pfx_pad[:], padP[:])
                fp_t = ph3.tile([P, NI], f32)
                nc.vector.scalar_tensor_tensor(
                    out=fp_t[:], in0=inclp[:], scalar=ones_col[:], in1=padP[:],
                    op0=Alu.is_equal, op1=Alu.mult,
                )
                lr = ph3.tile([P, NI], f32)
                nc.vector.memset(lr[:], 0.0)
                nc.sync.dma_start(lr[0 : P - 1, :], fp_t[1:P, :])
                nc.sync.dma_start(lr[P - 1 : P, 0 : NI - 1], fp_t[0:1, 1:NI])
                hbP = ph3.tile([P, NI], f32)
                nc.vector.tensor_max(hbP[:], hb0[:], lr[:])

                pfx_hb, inc_hb = excl_prefix(hbP, "hb")
                nc.vector.tensor_copy(segP[:], pfx_hb[:])
                nc.vector.tensor_copy(counts_sb[:], inc_hb[0:1, NI - 1 : NI])
                nc.sync.dma_start(
                    counts_d.rearrange("(a b) -> a b", a=1), counts_sb[:]
                )
                cb_ps = ps_seg.tile([P, 1], f32)
                nc.tensor.matmul(
                    cb_ps[:], ones_row[:], counts_sb[:], start=True, stop=True
                )
                counts_b = ph3.tile([P, 1], f32)
                nc.vector.tensor_copy(counts_b[:], cb_ps[:])
                shortP = ph3.tile([P, NI], f32)
                nc.vector.tensor_scalar(
                    shortP[:], iotaP[:], counts_b[:], None, op0=Alu.is_lt
                )
                nc.sync.dma_start(
                    short_d.rearrange("(i p) -> p i", p=P), shortP[:]
                )

            # ================= phase 4: pooling ===========================
            with (
                tc.tile_pool(name="ps_pa", bufs=2, space="PSUM") as ps_pa,
                tc.tile_pool(name="ps_pb", bufs=2, space="PSUM") as ps_pb,
                tc.tile_pool(name="barp", bufs=4) as barp,
                tc.tile_pool(name="outp", bufs=3) as outp,
                tc.tile_pool(name="iop", bufs=3) as iop,
            ):
                for s in range(NS):
                    segS = iop.tile([P, NI], f32)
                    nc.vector.tensor_scalar(
                        segS[:], segP[:], float(s * P), None, op0=Alu.subtract
                    )
                    pa = ps_pa.tile([P, 256], f32)
                    pb = ps_pb.tile([P, 258], f32)
                    for i in range(NI):
                        bar = barp.tile([P, P], pool_dt)
                        nc.vector.tensor_scalar(
                            bar[:], iotaB[:], segS[:, i : i + 1], None,
                            op0=Alu.is_equal,
                        )
                        nc.tensor.matmul(
                            pa[:], bar[:], h_pool[:, i, 0:256],
                            start=(i == 0), stop=(i == NI - 1),
                        )
                        nc.tensor.matmul(
                            pb[:], bar[:], h_pool[:, i, 256 : D + 2],
                            start=(i == 0), stop=(i == NI - 1),
                        )
                    cnt_eps = iop.tile([P, 1], f32)
                    nc.vector.tensor_scalar(
                        cnt_eps[:], pb[:, 256:257], 1e-9, None, op0=Alu.add
                    )
                    invc = iop.tile([P, 1], f32)
                    nc.vector.reciprocal(invc[:], cnt_eps[:])
                    ot = outp.tile([P, D], f32)
                    nc.vector.tensor_scalar(
                        ot[:, 0:256], pa[:], invc[:], None, op0=Alu.mult
                    )
                    nc.vector.tensor_scalar(
                        ot[:, 256:512], pb[:, 0:256], invc[:], None, op0=Alu.mult
                    )
                    nc.sync.dma_start(pooled_d[s * P : (s + 1) * P, :], ot[:])

    nc.finalize()
    return nc


def get_nc(pool_dt_name="float32"):
    if pool_dt_name not in _NC_CACHE:
        _NC_CACHE[pool_dt_name] = _build_nc(pool_dt_name)
    return _NC_CACHE[pool_dt_name]


def _host_loss(counts, n, target_boundary_counts):
    from scipy.special import gammaln

    c = counts.astype(np.float64)
    n64 = n.astype(np.float64)
    p = np.clip(
        (target_boundary_counts.astype(np.float32) / n.astype(np.float32)).astype(
            np.float64
        ),
        EPS,
        1.0 - EPS,
    )
    logpmf = (
        gammaln(n64 + 1.0)
        - gammaln(c + 1.0)
        - gammaln(n64 - c + 1.0)
        + c * np.log(p)
        + (n64 - c) * np.log1p(-p)
    )
    return np.float32(np.mean(-logpmf))


def kernel(hidden, attention_mask, target_boundary_counts, W1, b1, W2, b2, u,
           _run=None, pool_dt_name="float32"):
    from concourse.bass_utils import run_bass_kernel_spmd

    hidden = np.ascontiguousarray(hidden, np.float32)
    attention_mask = np.ascontiguousarray(attention_mask, np.float32)
    u = np.ascontiguousarray(u, np.float32)
    W1 = np.ascontiguousarray(W1, np.float32)
    b1 = np.ascontiguousarray(b1, np.float32)
    W2 = np.ascontiguousarray(W2, np.float32)
    b2 = np.ascontiguousarray(b2, np.float32)

    nc = get_nc(pool_dt_name)
    in_maps = [
        dict(
            hidden=hidden[i], mask=attention_mask[i], u=u[i],
            W1=W1, b1=b1, W2=W2, b2=b2,
        )
        for i in range(B)
    ]
    if _run is None:
        results = run_bass_kernel_spmd(nc, in_maps, list(range(B))).results
    else:
        results = _run(nc, in_maps)

    pooled = np.stack([results[i]["pooled"] for i in range(B)])
    short_mask = np.stack([results[i]["short_mask"] for i in range(B)])
    counts = np.array([results[i]["counts"][0] for i in range(B)], np.float32)

    n = attention_mask.sum(axis=1, dtype=np.float32)
    loss = _host_loss(counts, n, target_boundary_counts)
    hb_sum = np.float32(counts.sum())
    n_sum = np.float32(n.sum())
    return pooled, loss, hb_sum, n_sum, short_mask
